# revision 8
# baseline (speedup 1.0000x reference)
"""2-layer GCN (GCNConv x2) on 8 trn2 NeuronCores.

Strategy (node-parallel):
  - nodes sharded 12500/core; phase A computes p1 = x @ W1 per shard
    (PE transpose of x tiles + matmul), writes fp16 table shard.
  - AllGather table1 to every core's HBM.
  - L1 aggregation: edges (dst-sorted, chunked 128 slots / <=8 dst nodes)
    per chunk: indirect-DMA row gather from table1 + matmul with a narrow
    norm matrix M [128 x 8] accumulating transposed output columns in PSUM;
    per 64-chunk group: ACT relu(psum + b1) -> h1T fp16.
  - h2T = W2^T @ h1T per group (PE), transposed back to row-major fp16
    table2 shard; AllGather table2.
  - L2 aggregation: same chunk structure (same graph/norms -> M reused),
    gathers from table2, outputs psum [64 x cols] copied to f32 out.
  - host: reorder slotted transposed output to [N, 64], add b2.
"""
import numpy as np

N_NODES = 100000
N_CORES = 8
NC_N = N_NODES // N_CORES          # 12500
NC_PAD = 12544                     # 98 * 128, zero-padded phase-A shard
IN_CH, HID, OUT_CH = 256, 128, 64
W = 8                              # dst nodes per chunk (psum col window)
GRP = 64                           # chunks per psum group (512 cols)

_CACHE = {}


def _preprocess(edge_index):
    src = np.asarray(edge_index[0], dtype=np.int64)
    dst = np.asarray(edge_index[1], dtype=np.int64)
    loop = np.arange(N_NODES, dtype=np.int64)
    s = np.concatenate([src, loop])
    d = np.concatenate([dst, loop])
    deg = np.bincount(d, minlength=N_NODES).astype(np.float64)
    dinv = 1.0 / np.sqrt(deg)
    norm = (dinv[s] * dinv[d]).astype(np.float32)

    slot_global = np.zeros(N_NODES, dtype=np.int64)  # slot id within owner core
    per_core = []
    for c in range(N_CORES):
        lo, hi = c * NC_N, (c + 1) * NC_N
        m = (d >= lo) & (d < hi)
        sc, dc, nc_ = s[m], d[m], norm[m]
        order = np.argsort(dc, kind="stable")
        sc, dc, nc_ = sc[order], dc[order], nc_[order]
        counts = np.bincount(dc - lo, minlength=NC_N)
        # greedy chunking: <=128 slots, <=W nodes per chunk
        chunk_of_node = np.zeros(NC_N, np.int64)
        col_of_node = np.zeros(NC_N, np.int64)
        ch, slots, nodes = 0, 0, 0
        for v in range(NC_N):
            k = int(counts[v])
            if nodes == W or slots + k > 128:
                ch += 1
                slots, nodes = 0, 0
            chunk_of_node[v] = ch
            col_of_node[v] = nodes
            slots += k
            nodes += 1
        nch_c = ch + 1
        slot_global[lo:hi] = chunk_of_node * W + col_of_node
        per_core.append((sc, dc, nc_, counts, chunk_of_node, col_of_node, nch_c))

    NCH = max(pc[6] for pc in per_core)
    NCH = ((NCH + GRP - 1) // GRP) * GRP  # round up so S is consistent everywhere
    S = NCH * W
    # table1 row of node v: owner*NC_PAD + (v - owner*NC_N)
    own = np.arange(N_NODES) // NC_N
    t1row = own * NC_PAD + (np.arange(N_NODES) - own * NC_N)
    # table2 row of node v: owner*S + slot_global[v]
    t2row = own * S + slot_global

    cores = []
    for c in range(N_CORES):
        sc, dc, nc_, counts, chunk_of_node, col_of_node, nch_c = per_core[c]
        E = len(sc)
        # slot position of each edge: edges sorted by dst; within chunk,
        # consecutive by node.  compute per-chunk running slot offsets.
        node_first_edge = np.zeros(NC_N + 1, np.int64)
        np.cumsum(counts, out=node_first_edge[1:])
        idx1 = np.zeros((NCH, 128), np.int32)
        mval = np.zeros((NCH, 128, W), np.float16)
        idx2 = np.zeros((NCH, 128), np.int32)
        slot_in_chunk = np.zeros(NCH, np.int64)
        for v in range(NC_N):
            k = int(counts[v])
            ch = int(chunk_of_node[v])
            col = int(col_of_node[v])
            if k == 0:
                continue
            e0 = int(node_first_edge[v])
            p0 = int(slot_in_chunk[ch])
            srcs = sc[e0 : e0 + k]
            idx1[ch, p0 : p0 + k] = t1row[srcs]
            idx2[ch, p0 : p0 + k] = t2row[srcs]
            mval[ch, p0 : p0 + k, col] = nc_[e0 : e0 + k]
            slot_in_chunk[ch] = p0 + k
        cores.append(
            dict(
                idx1=np.ascontiguousarray(idx1.T),            # [128, NCH]
                idx2=np.ascontiguousarray(idx2.T),            # [128, NCH]
                m=np.ascontiguousarray(mval.transpose(1, 0, 2).reshape(128, NCH * W)),
            )
        )
    return cores, NCH, S, slot_global


def _build_kernel(NCH, S):
    import concourse.bass as bass
    import concourse.mybir as mybir
    from concourse import tile
    from concourse.masks import make_identity

    NGRP = (NCH + GRP - 1) // GRP
    assert NCH % GRP == 0
    f16, f32, i32 = mybir.dt.float16, mybir.dt.float32, mybir.dt.int32

    nc = bass.Bass(num_devices=N_CORES)
    x_in = nc.dram_tensor("x", [NC_PAD, IN_CH], f32, kind="ExternalInput")
    w1_in = nc.dram_tensor("w1", [IN_CH, HID], f32, kind="ExternalInput")
    w2_in = nc.dram_tensor("w2h", [HID, OUT_CH], f16, kind="ExternalInput")
    b1_in = nc.dram_tensor("b1col", [HID, 1], f32, kind="ExternalInput")
    idx1_in = nc.dram_tensor("idx1", [128, NCH], i32, kind="ExternalInput")
    idx2_in = nc.dram_tensor("idx2", [128, NCH], i32, kind="ExternalInput")
    m_in = nc.dram_tensor("m", [128, NCH * W], f16, kind="ExternalInput")
    out_t = nc.dram_tensor("outT", [OUT_CH, S], f32, kind="ExternalOutput")

    t1_local = nc.dram_tensor("t1_local", [NC_PAD, HID], f16, kind="Internal")
    table1 = nc.dram_tensor(
        "table1", [N_CORES * NC_PAD, HID], f16, kind="Internal", addr_space="Shared"
    )
    t2_local = nc.dram_tensor("t2_local", [S, OUT_CH], f16, kind="Internal")
    table2 = nc.dram_tensor(
        "table2", [N_CORES * S, OUT_CH], f16, kind="Internal", addr_space="Shared"
    )

    with tile.TileContext(nc) as tc:
        with (
            tc.tile_pool(name="const", bufs=1) as cpool,
            tc.tile_pool(name="xin", bufs=3) as xpool,
            tc.tile_pool(name="xt", bufs=3) as xtpool,
            tc.tile_pool(name="stage", bufs=4) as spool,
            tc.tile_pool(name="g", bufs=24) as gpool,
            tc.tile_pool(name="mi", bufs=3) as mpool,
            tc.tile_pool(name="h1", bufs=3) as hpool,
            tc.tile_pool(name="psum", bufs=6, space="PSUM") as pspool,
        ):
            ident = cpool.tile([128, 128], f32)
            make_identity(nc, ident[:])
            identh = cpool.tile([128, 128], f16, name="identh")
            make_identity(nc, identh[:])
            # W1 [256,128] does not fit partitions; store as two k-tiles
            w1a = cpool.tile([128, HID], f32, name="w1a")
            w1b = cpool.tile([128, HID], f32, name="w1b")
            nc.sync.dma_start(out=w1a[:], in_=w1_in[0:128, :])
            nc.sync.dma_start(out=w1b[:], in_=w1_in[128:256, :])
            w2_sb = cpool.tile([HID, OUT_CH], f16, name="w2sb")
            nc.sync.dma_start(out=w2_sb[:], in_=w2_in[:])
            b1_sb = cpool.tile([HID, 1], f32, name="b1sb")
            nc.sync.dma_start(out=b1_sb[:], in_=b1_in[:])

            # ---------- phase A: p1 = x @ W1 per 128-node tile ----------
            for t in range(NC_PAD // 128):
                xt = xpool.tile([128, IN_CH], f32, tag="xin")
                nc.sync.dma_start(out=xt[:], in_=x_in[t * 128 : (t + 1) * 128, :])
                pst = pspool.tile([128, 512], f32, tag="ps")
                nc.tensor.transpose(out=pst[:, 0:128], in_=xt[:, 0:128], identity=ident[:])
                nc.tensor.transpose(out=pst[:, 128:256], in_=xt[:, 128:256], identity=ident[:])
                xT0 = xtpool.tile([128, 128], f32, tag="xt0")
                xT1 = xtpool.tile([128, 128], f32, tag="xt1")
                nc.scalar.copy(out=xT0[:], in_=pst[:, 0:128])
                nc.scalar.copy(out=xT1[:], in_=pst[:, 128:256])
                psp = pspool.tile([128, 512], f32, tag="ps")
                nc.tensor.matmul(out=psp[:, 0:HID], lhsT=xT0[:], rhs=w1a[:], start=True, stop=False)
                nc.tensor.matmul(out=psp[:, 0:HID], lhsT=xT1[:], rhs=w1b[:], start=False, stop=True)
                p1t = spool.tile([128, HID], f16, tag="p1")
                nc.scalar.copy(out=p1t[:], in_=psp[:, 0:HID])
                nc.sync.dma_start(out=t1_local[t * 128 : (t + 1) * 128, :], in_=p1t[:])

            # ---------- allgather table1 ----------
            nc.gpsimd.collective_compute(
                "AllGather",
                mybir.AluOpType.bypass,
                replica_groups=[list(range(N_CORES))],
                ins=[t1_local[:]],
                outs=[table1[:]],
            )

            # ---------- L1 aggregation + L2 prep ----------
            for g in range(NGRP):
                m_t = mpool.tile([128, GRP * W], f16, tag="m")
                nc.sync.dma_start(out=m_t[:], in_=m_in[:, g * GRP * W : (g + 1) * GRP * W])
                i_t = mpool.tile([128, GRP], i32, tag="i1")
                nc.sync.dma_start(out=i_t[:], in_=idx1_in[:, g * GRP : (g + 1) * GRP])
                ps = pspool.tile([128, 512], f32, tag="ps")
                for k in range(GRP):
                    gt = gpool.tile([128, HID], f16, tag="g")
                    nc.gpsimd.indirect_dma_start(
                        out=gt[:],
                        out_offset=None,
                        in_=table1[:],
                        in_offset=bass.IndirectOffsetOnAxis(ap=i_t[:, k : k + 1], axis=0),
                    )
                    nc.tensor.matmul(
                        out=ps[:, k * W : (k + 1) * W],
                        lhsT=gt[:],
                        rhs=m_t[:, k * W : (k + 1) * W],
                        start=True,
                        stop=True,
                    )
                h1 = hpool.tile([128, 512], f16, tag="h1")
                nc.scalar.activation(
                    out=h1[:], in_=ps[:],
                    func=mybir.ActivationFunctionType.Relu,
                    bias=b1_sb[:, :1], scale=1.0,
                )
                # h2T = W2^T @ h1T   [64 x 512]
                ps2 = pspool.tile([128, 512], f32, tag="ps")
                nc.tensor.matmul(out=ps2[:OUT_CH, :], lhsT=w2_sb[:], rhs=h1[:], start=True, stop=True)
                g2s = spool.tile([OUT_CH, 512], f16, tag="g2s")
                nc.scalar.copy(out=g2s[:], in_=ps2[:OUT_CH, :])
                # transpose to row-major [512 x 64] in 4 sub-tiles
                for q in range(4):
                    ps3 = pspool.tile([128, 1024], f16, tag="psh", bufs=2)
                    nc.tensor.transpose(
                        out=ps3[:, :OUT_CH],
                        in_=g2s[:, q * 128 : (q + 1) * 128],
                        identity=identh[:OUT_CH, :OUT_CH],
                    )
                    t2t = spool.tile([128, OUT_CH], f16, tag="t2t")
                    nc.scalar.copy(out=t2t[:], in_=ps3[:, :OUT_CH])
                    r0 = g * 512 + q * 128
                    nc.sync.dma_start(out=t2_local[r0 : r0 + 128, :], in_=t2t[:])

            # ---------- allgather table2 ----------
            nc.gpsimd.collective_compute(
                "AllGather",
                mybir.AluOpType.bypass,
                replica_groups=[list(range(N_CORES))],
                ins=[t2_local[:]],
                outs=[table2[:]],
            )

            # ---------- L2 aggregation ----------
            for g in range(NGRP):
                m_t = mpool.tile([128, GRP * W], f16, tag="m")
                nc.sync.dma_start(out=m_t[:], in_=m_in[:, g * GRP * W : (g + 1) * GRP * W])
                i_t = mpool.tile([128, GRP], i32, tag="i2")
                nc.sync.dma_start(out=i_t[:], in_=idx2_in[:, g * GRP : (g + 1) * GRP])
                ps = pspool.tile([128, 512], f32, tag="ps")
                for k in range(GRP):
                    gt2 = gpool.tile([128, OUT_CH], f16, tag="g2")
                    nc.gpsimd.indirect_dma_start(
                        out=gt2[:],
                        out_offset=None,
                        in_=table2[:],
                        in_offset=bass.IndirectOffsetOnAxis(ap=i_t[:, k : k + 1], axis=0),
                    )
                    nc.tensor.matmul(
                        out=ps[:OUT_CH, k * W : (k + 1) * W],
                        lhsT=gt2[:],
                        rhs=m_t[:, k * W : (k + 1) * W],
                        start=True,
                        stop=True,
                    )
                osb = spool.tile([OUT_CH, 512], f32, tag="osb")
                nc.scalar.copy(out=osb[:], in_=ps[:OUT_CH, :])
                nc.sync.dma_start(out=out_t[:, g * 512 : (g + 1) * 512], in_=osb[:])

    from tile_patch_embedded import split_multi_waits

    split_multi_waits(nc)
    return nc


# --- embedded copy of the walrus multi-wait workaround (self-contained) ---
import sys as _sys
import types as _types

_tp_src = '''
import concourse.mybir as mybir

def split_multi_waits(nc, max_waits=1):
    n_split = 0
    for fn in nc.m.functions:
        for blk in fn.blocks:
            insts = blk.instructions
            i = 0
            while i < len(insts):
                inst = insts[i]
                si = inst.sync_info
                waits = list(si.on_wait) if si is not None else []
                if len(waits) > max_waits:
                    keep = waits[:max_waits]
                    extra = waits[max_waits:]
                    si.on_wait = keep
                    new_nops = []
                    for k in range(0, len(extra), max_waits):
                        nop = mybir.InstNoOp(
                            name=f"{inst.name}-xw{k}",
                            sync_info=mybir.SyncInfo(
                                on_wait=extra[k : k + max_waits], on_update=[]
                            ),
                            bass_nofuse=True,
                            engine=inst.engine,
                        )
                        new_nops.append(nop)
                        nc.register_instruction(nop, overwrite=True)
                    insts[i:i] = new_nops
                    i += len(new_nops)
                    n_split += 1
                i += 1
    return n_split
'''
_tp_mod = _types.ModuleType("tile_patch_embedded")
exec(_tp_src, _tp_mod.__dict__)
_sys.modules["tile_patch_embedded"] = _tp_mod


def kernel(x, edge_index, W1, b1, W2, b2):
    from concourse.bass_utils import run_bass_kernel_spmd

    x = np.asarray(x, dtype=np.float32)
    W1 = np.asarray(W1, dtype=np.float32)
    W2 = np.asarray(W2, dtype=np.float32)
    b1 = np.asarray(b1, dtype=np.float32)
    b2 = np.asarray(b2, dtype=np.float32)

    ekey = hash(np.asarray(edge_index)[:, ::997].tobytes())
    if ekey in _CACHE:
        cores, NCH, S, slot_global, nc = _CACHE[ekey]
    else:
        cores, NCH, S, slot_global = _preprocess(edge_index)
        nc = _build_kernel(NCH, S)
        _CACHE[ekey] = (cores, NCH, S, slot_global, nc)

    b1col = np.ascontiguousarray(b1.reshape(HID, 1))
    w2h = W2.astype(np.float16)
    in_maps = []
    for c in range(N_CORES):
        xs = np.zeros((NC_PAD, IN_CH), np.float32)
        xs[:NC_N] = x[c * NC_N : (c + 1) * NC_N]
        in_maps.append(
            dict(
                x=xs, w1=W1, w2h=w2h, b1col=b1col,
                idx1=cores[c]["idx1"], idx2=cores[c]["idx2"], m=cores[c]["m"],
            )
        )
    res = run_bass_kernel_spmd(nc, in_maps, core_ids=list(range(N_CORES)))
    # assemble: out[v] = outT[core(v)][:, slot_global(v)] + b2
    outs = np.stack([res.results[c]["outT"] for c in range(N_CORES)])  # [8, 64, S]
    own = np.arange(N_NODES) // NC_N
    out = outs[own, :, slot_global].astype(np.float32)  # [N, 64]
    out = out + b2[None, :]
    return out


# revision 9
# speedup vs baseline: 621.3086x; 621.3086x over previous
"""2-layer GCN (GCNConv x2) on 8 trn2 NeuronCores.

Strategy (node-parallel):
  - nodes sharded 12500/core; phase A computes p1 = x @ W1 per shard
    (PE transpose of x tiles + matmul), writes fp16 table shard.
  - AllGather table1 to every core's HBM.
  - L1 aggregation: edges (dst-sorted, chunked 128 slots / <=8 dst nodes)
    per chunk: indirect-DMA row gather from table1 + matmul with a narrow
    norm matrix M [128 x 8] accumulating transposed output columns in PSUM;
    per 64-chunk group: ACT relu(psum + b1) -> h1T fp16.
  - h2T = W2^T @ h1T per group (PE), transposed back to row-major fp16
    table2 shard; AllGather table2.
  - L2 aggregation: same chunk structure (same graph/norms -> M reused),
    gathers from table2, outputs psum [64 x cols] copied to f32 out.
  - host: reorder slotted transposed output to [N, 64], add b2.
"""
import numpy as np

N_NODES = 100000
N_CORES = 8
NC_N = N_NODES // N_CORES          # 12500
NC_PAD = 12544                     # 98 * 128, zero-padded phase-A shard
IN_CH, HID, OUT_CH = 256, 128, 64
W = 8                              # dst nodes per chunk (psum col window)
GRP = 64                           # chunks per psum group (512 cols)

_CACHE = {}


def _preprocess(edge_index):
    src = np.asarray(edge_index[0], dtype=np.int64)
    dst = np.asarray(edge_index[1], dtype=np.int64)
    loop = np.arange(N_NODES, dtype=np.int64)
    s = np.concatenate([src, loop])
    d = np.concatenate([dst, loop])
    deg = np.bincount(d, minlength=N_NODES).astype(np.float64)
    dinv = 1.0 / np.sqrt(deg)
    norm = (dinv[s] * dinv[d]).astype(np.float32)

    slot_global = np.zeros(N_NODES, dtype=np.int64)  # slot id within owner core
    per_core = []
    for c in range(N_CORES):
        lo, hi = c * NC_N, (c + 1) * NC_N
        m = (d >= lo) & (d < hi)
        sc, dc, nc_ = s[m], d[m], norm[m]
        order = np.argsort(dc, kind="stable")
        sc, dc, nc_ = sc[order], dc[order], nc_[order]
        counts = np.bincount(dc - lo, minlength=NC_N)
        # greedy chunking: <=128 slots, <=W nodes per chunk
        chunk_of_node = np.zeros(NC_N, np.int64)
        col_of_node = np.zeros(NC_N, np.int64)
        ch, slots, nodes = 0, 0, 0
        for v in range(NC_N):
            k = int(counts[v])
            if nodes == W or slots + k > 128:
                ch += 1
                slots, nodes = 0, 0
            chunk_of_node[v] = ch
            col_of_node[v] = nodes
            slots += k
            nodes += 1
        nch_c = ch + 1
        slot_global[lo:hi] = chunk_of_node * W + col_of_node
        per_core.append((sc, dc, nc_, counts, chunk_of_node, col_of_node, nch_c))

    NCH = max(pc[6] for pc in per_core)
    NCH = ((NCH + GRP - 1) // GRP) * GRP  # round up so S is consistent everywhere
    S = NCH * W
    # table1 row of node v: owner*NC_PAD + (v - owner*NC_N)
    own = np.arange(N_NODES) // NC_N
    t1row = own * NC_PAD + (np.arange(N_NODES) - own * NC_N)
    # table2 row of node v: owner*S + slot_global[v]
    t2row = own * S + slot_global

    cores = []
    for c in range(N_CORES):
        sc, dc, nc_, counts, chunk_of_node, col_of_node, nch_c = per_core[c]
        E = len(sc)
        # slot position of each edge: edges sorted by dst; within chunk,
        # consecutive by node.  compute per-chunk running slot offsets.
        node_first_edge = np.zeros(NC_N + 1, np.int64)
        np.cumsum(counts, out=node_first_edge[1:])
        idx1 = np.zeros((NCH, 128), np.int32)
        mval = np.zeros((NCH, 128, W), np.float16)
        idx2 = np.zeros((NCH, 128), np.int32)
        slot_in_chunk = np.zeros(NCH, np.int64)
        for v in range(NC_N):
            k = int(counts[v])
            ch = int(chunk_of_node[v])
            col = int(col_of_node[v])
            if k == 0:
                continue
            e0 = int(node_first_edge[v])
            p0 = int(slot_in_chunk[ch])
            srcs = sc[e0 : e0 + k]
            idx1[ch, p0 : p0 + k] = t1row[srcs]
            idx2[ch, p0 : p0 + k] = t2row[srcs]
            mval[ch, p0 : p0 + k, col] = nc_[e0 : e0 + k]
            slot_in_chunk[ch] = p0 + k
        cores.append(
            dict(
                idx1=np.ascontiguousarray(idx1.T),            # [128, NCH]
                idx2=np.ascontiguousarray(idx2.T),            # [128, NCH]
                m=np.ascontiguousarray(mval.transpose(1, 0, 2).reshape(128, NCH * W)),
            )
        )
    return cores, NCH, S, slot_global


def _build_kernel(NCH, S):
    import concourse.bass as bass
    import concourse.mybir as mybir
    from concourse import tile
    from concourse.masks import make_identity

    NGRP = (NCH + GRP - 1) // GRP
    assert NCH % GRP == 0
    f16, f32, i32 = mybir.dt.float16, mybir.dt.float32, mybir.dt.int32

    nc = bass.Bass(num_devices=N_CORES, num_swdge_queues=4)
    x_in = nc.dram_tensor("x", [NC_PAD, IN_CH], f32, kind="ExternalInput")
    w1_in = nc.dram_tensor("w1", [IN_CH, HID], f32, kind="ExternalInput")
    w2_in = nc.dram_tensor("w2h", [HID, OUT_CH], f16, kind="ExternalInput")
    b1_in = nc.dram_tensor("b1col", [HID, 1], f32, kind="ExternalInput")
    idx1_in = nc.dram_tensor("idx1", [128, NCH], i32, kind="ExternalInput")
    idx2_in = nc.dram_tensor("idx2", [128, NCH], i32, kind="ExternalInput")
    m_in = nc.dram_tensor("m", [128, NCH * W], f16, kind="ExternalInput")
    out_t = nc.dram_tensor("outT", [OUT_CH, S], f32, kind="ExternalOutput")

    t1_local = nc.dram_tensor("t1_local", [NC_PAD, HID], f16, kind="Internal")
    table1 = nc.dram_tensor(
        "table1", [N_CORES * NC_PAD, HID], f16, kind="Internal", addr_space="Shared"
    )
    t2_local = nc.dram_tensor("t2_local", [S, OUT_CH], f16, kind="Internal")
    table2 = nc.dram_tensor(
        "table2", [N_CORES * S, OUT_CH], f16, kind="Internal", addr_space="Shared"
    )

    with tile.TileContext(nc) as tc:
        with (
            tc.tile_pool(name="const", bufs=1) as cpool,
            tc.tile_pool(name="xin", bufs=3) as xpool,
            tc.tile_pool(name="xt", bufs=3) as xtpool,
            tc.tile_pool(name="stage", bufs=4) as spool,
            tc.tile_pool(name="g", bufs=24) as gpool,
            tc.tile_pool(name="mi", bufs=3) as mpool,
            tc.tile_pool(name="h1", bufs=3) as hpool,
            tc.tile_pool(name="psum", bufs=6, space="PSUM") as pspool,
        ):
            ident = cpool.tile([128, 128], f32)
            make_identity(nc, ident[:])
            identh = cpool.tile([128, 128], f16, name="identh")
            make_identity(nc, identh[:])
            # W1 [256,128] does not fit partitions; store as two k-tiles
            w1a = cpool.tile([128, HID], f32, name="w1a")
            w1b = cpool.tile([128, HID], f32, name="w1b")
            nc.sync.dma_start(out=w1a[:], in_=w1_in[0:128, :])
            nc.sync.dma_start(out=w1b[:], in_=w1_in[128:256, :])
            w2_sb = cpool.tile([HID, OUT_CH], f16, name="w2sb")
            nc.sync.dma_start(out=w2_sb[:], in_=w2_in[:])
            b1_sb = cpool.tile([HID, 1], f32, name="b1sb")
            nc.sync.dma_start(out=b1_sb[:], in_=b1_in[:])

            # ---------- phase A: p1 = x @ W1 per 128-node tile ----------
            for t in range(NC_PAD // 128):
                xt = xpool.tile([128, IN_CH], f32, tag="xin")
                nc.sync.dma_start(out=xt[:], in_=x_in[t * 128 : (t + 1) * 128, :])
                pst = pspool.tile([128, 512], f32, tag="ps")
                nc.tensor.transpose(out=pst[:, 0:128], in_=xt[:, 0:128], identity=ident[:])
                nc.tensor.transpose(out=pst[:, 128:256], in_=xt[:, 128:256], identity=ident[:])
                xT0 = xtpool.tile([128, 128], f32, tag="xt0")
                xT1 = xtpool.tile([128, 128], f32, tag="xt1")
                nc.scalar.copy(out=xT0[:], in_=pst[:, 0:128])
                nc.scalar.copy(out=xT1[:], in_=pst[:, 128:256])
                psp = pspool.tile([128, 512], f32, tag="ps")
                nc.tensor.matmul(out=psp[:, 0:HID], lhsT=xT0[:], rhs=w1a[:], start=True, stop=False)
                nc.tensor.matmul(out=psp[:, 0:HID], lhsT=xT1[:], rhs=w1b[:], start=False, stop=True)
                p1t = spool.tile([128, HID], f16, tag="p1")
                nc.scalar.copy(out=p1t[:], in_=psp[:, 0:HID])
                nc.sync.dma_start(out=t1_local[t * 128 : (t + 1) * 128, :], in_=p1t[:])

            # ---------- allgather table1 ----------
            nc.gpsimd.collective_compute(
                "AllGather",
                mybir.AluOpType.bypass,
                replica_groups=[list(range(N_CORES))],
                ins=[t1_local[:]],
                outs=[table1[:]],
            )

            # ---------- L1 aggregation + L2 prep ----------
            for g in range(NGRP):
                m_t = mpool.tile([128, GRP * W], f16, tag="m")
                nc.sync.dma_start(out=m_t[:], in_=m_in[:, g * GRP * W : (g + 1) * GRP * W])
                i_t = mpool.tile([128, GRP], i32, tag="i1")
                nc.sync.dma_start(out=i_t[:], in_=idx1_in[:, g * GRP : (g + 1) * GRP])
                ps = pspool.tile([128, 512], f32, tag="ps")
                for k in range(GRP):
                    gt = gpool.tile([128, HID], f16, tag="g")
                    bi = nc.gpsimd.indirect_dma_start(
                        out=gt[:],
                        out_offset=None,
                        in_=table1[:],
                        in_offset=bass.IndirectOffsetOnAxis(ap=i_t[:, k : k + 1], axis=0),
                    )
                    bi.ins.queue = f"qPoolDynamic{(k % 4) or ''}"
                    nc.tensor.matmul(
                        out=ps[:, k * W : (k + 1) * W],
                        lhsT=gt[:],
                        rhs=m_t[:, k * W : (k + 1) * W],
                        start=True,
                        stop=True,
                    )
                h1 = hpool.tile([128, 512], f16, tag="h1")
                nc.scalar.activation(
                    out=h1[:], in_=ps[:],
                    func=mybir.ActivationFunctionType.Relu,
                    bias=b1_sb[:, :1], scale=1.0,
                )
                # h2T = W2^T @ h1T   [64 x 512]
                ps2 = pspool.tile([128, 512], f32, tag="ps")
                nc.tensor.matmul(out=ps2[:OUT_CH, :], lhsT=w2_sb[:], rhs=h1[:], start=True, stop=True)
                g2s = spool.tile([OUT_CH, 512], f16, tag="g2s")
                nc.scalar.copy(out=g2s[:], in_=ps2[:OUT_CH, :])
                # transpose to row-major [512 x 64] in 4 sub-tiles
                for q in range(4):
                    ps3 = pspool.tile([128, 1024], f16, tag="psh", bufs=2)
                    nc.tensor.transpose(
                        out=ps3[:, :OUT_CH],
                        in_=g2s[:, q * 128 : (q + 1) * 128],
                        identity=identh[:OUT_CH, :OUT_CH],
                    )
                    t2t = spool.tile([128, OUT_CH], f16, tag="t2t")
                    nc.scalar.copy(out=t2t[:], in_=ps3[:, :OUT_CH])
                    r0 = g * 512 + q * 128
                    nc.sync.dma_start(out=t2_local[r0 : r0 + 128, :], in_=t2t[:])

            # ---------- allgather table2 ----------
            nc.gpsimd.collective_compute(
                "AllGather",
                mybir.AluOpType.bypass,
                replica_groups=[list(range(N_CORES))],
                ins=[t2_local[:]],
                outs=[table2[:]],
            )

            # ---------- L2 aggregation ----------
            for g in range(NGRP):
                m_t = mpool.tile([128, GRP * W], f16, tag="m")
                nc.sync.dma_start(out=m_t[:], in_=m_in[:, g * GRP * W : (g + 1) * GRP * W])
                i_t = mpool.tile([128, GRP], i32, tag="i2")
                nc.sync.dma_start(out=i_t[:], in_=idx2_in[:, g * GRP : (g + 1) * GRP])
                ps = pspool.tile([128, 512], f32, tag="ps")
                for k in range(GRP):
                    gt2 = gpool.tile([128, OUT_CH], f16, tag="g2")
                    bi2 = nc.gpsimd.indirect_dma_start(
                        out=gt2[:],
                        out_offset=None,
                        in_=table2[:],
                        in_offset=bass.IndirectOffsetOnAxis(ap=i_t[:, k : k + 1], axis=0),
                    )
                    bi2.ins.queue = f"qPoolDynamic{(k % 4) or ''}"
                    nc.tensor.matmul(
                        out=ps[:OUT_CH, k * W : (k + 1) * W],
                        lhsT=gt2[:],
                        rhs=m_t[:, k * W : (k + 1) * W],
                        start=True,
                        stop=True,
                    )
                osb = spool.tile([OUT_CH, 512], f32, tag="osb")
                nc.scalar.copy(out=osb[:], in_=ps[:OUT_CH, :])
                nc.sync.dma_start(out=out_t[:, g * 512 : (g + 1) * 512], in_=osb[:])

    from tile_patch_embedded import split_multi_waits

    split_multi_waits(nc)
    return nc


# --- embedded copy of the walrus multi-wait workaround (self-contained) ---
import sys as _sys
import types as _types

_tp_src = '''
import concourse.mybir as mybir

def split_multi_waits(nc, max_waits=1):
    n_split = 0
    for fn in nc.m.functions:
        for blk in fn.blocks:
            insts = blk.instructions
            i = 0
            while i < len(insts):
                inst = insts[i]
                si = inst.sync_info
                waits = list(si.on_wait) if si is not None else []
                if len(waits) > max_waits:
                    keep = waits[:max_waits]
                    extra = waits[max_waits:]
                    si.on_wait = keep
                    new_nops = []
                    for k in range(0, len(extra), max_waits):
                        nop = mybir.InstNoOp(
                            name=f"{inst.name}-xw{k}",
                            sync_info=mybir.SyncInfo(
                                on_wait=extra[k : k + max_waits], on_update=[]
                            ),
                            bass_nofuse=True,
                            engine=inst.engine,
                        )
                        new_nops.append(nop)
                        nc.register_instruction(nop, overwrite=True)
                    insts[i:i] = new_nops
                    i += len(new_nops)
                    n_split += 1
                i += 1
    return n_split
'''
_tp_mod = _types.ModuleType("tile_patch_embedded")
exec(_tp_src, _tp_mod.__dict__)
_sys.modules["tile_patch_embedded"] = _tp_mod


def kernel(x, edge_index, W1, b1, W2, b2):
    from concourse.bass_utils import run_bass_kernel_spmd

    x = np.asarray(x, dtype=np.float32)
    W1 = np.asarray(W1, dtype=np.float32)
    W2 = np.asarray(W2, dtype=np.float32)
    b1 = np.asarray(b1, dtype=np.float32)
    b2 = np.asarray(b2, dtype=np.float32)

    ekey = hash(np.asarray(edge_index)[:, ::997].tobytes())
    if ekey in _CACHE:
        cores, NCH, S, slot_global, nc = _CACHE[ekey]
    else:
        cores, NCH, S, slot_global = _preprocess(edge_index)
        nc = _build_kernel(NCH, S)
        _CACHE[ekey] = (cores, NCH, S, slot_global, nc)

    b1col = np.ascontiguousarray(b1.reshape(HID, 1))
    w2h = W2.astype(np.float16)
    in_maps = []
    for c in range(N_CORES):
        xs = np.zeros((NC_PAD, IN_CH), np.float32)
        xs[:NC_N] = x[c * NC_N : (c + 1) * NC_N]
        in_maps.append(
            dict(
                x=xs, w1=W1, w2h=w2h, b1col=b1col,
                idx1=cores[c]["idx1"], idx2=cores[c]["idx2"], m=cores[c]["m"],
            )
        )
    res = run_bass_kernel_spmd(nc, in_maps, core_ids=list(range(N_CORES)))
    # assemble: out[v] = outT[core(v)][:, slot_global(v)] + b2
    outs = np.stack([res.results[c]["outT"] for c in range(N_CORES)])  # [8, 64, S]
    own = np.arange(N_NODES) // NC_N
    out = outs[own, :, slot_global].astype(np.float32)  # [N, 64]
    out = out + b2[None, :]
    return out
